# revision 39
# baseline (speedup 1.0000x reference)
"""Trainium2 Bass kernel for nn_Attention_81458349736162.

Batch-parallel over the 8 NeuronCores: each core owns B/8 = 4 batches and
runs the full attention + MLP for them; no collectives are needed.

Math (per batch b):
  ua_b = Ua @ normal_b + Ua_b ;  c_b = Wa_b - ua_b              (host)
  QR:  Wa = Q R  =>  dist_n^2 = ||Wa d_n + c_b||^2 = ||R d_n + c~_b||^2
     with R upper-triangular (host QR) and c~_b = Q^T c_b (host).

All-fp8e4 datapath (validated on host: rel_fro ~1.7e-3 vs 2e-2 budget):
  - Pool (gpsimd) casts the f32 defect stream to fp8e4.
  - PE transposes d tiles (fp8), ScalarE copies PSUM->SBUF.
  - z = R d accumulates in PSUM via 4 triangular fp8 matmuls (no seed
    matmul: the +c~ is fused into the distance reduce).
  - dist2 = sum_h (z + c~)^2 via ONE custom DVE op (sq(Src0+Src1) with
    add-accumulate), reading z from PSUM and a broadcast c~ row from SBUF.
  - dist  = exp(0.5*ln(dist2)); e = exp(dist - 20)  (ScalarE, f32)
  - ctx   = (sum_n e_n d_n) / sum(e) via fp8 DoubleRow matmuls that pair
    two 128-defect tiles per instruction (K=256).
  - out   = W2 @ relu(W1 @ [ctx, glob] + b1) + b2   (f32, tiny)
"""

import os
import numpy as np

B, N, H, OUT, MID = 32, 4096, 512, 5, 128
NCORES = 8
BLOC = B // NCORES          # batches per core
P = 128                     # partitions
T = N // P                  # 32 n-tiles per batch
HC = H // P                 # 4 h-chunks
MB = 2048                   # free-dim elems per DMA group (4 tiles of 512)
G = (T * H) // MB           # 8 DMA groups per batch
SHIFT = 20.0                # softmax shift constant (dist ~ 18.5 +- 1)

# ---- feature flags (fallbacks if a fast path fails on HW) ----
USE_CUSTOM_SQ = True        # fused sq(z + c) reduce on DVE; else seed matmul
CTX_DOUBLE_ROW = True       # fp8 DoubleRow 2-tile context; else plain fp8
CAST_DMA = True             # gpsimd SWDGE DMA casts f32->fp8 in-flight
CAST_ENGINE = "gpsimd"      # cast engine when CAST_DMA is False
DT_COPY_ENGINE = "scalar"   # engine for the dT PSUM->SBUF copy
# R-matmul mode: "drswi" = uint16 pair-transpose + DoubleRowSwInterleave
# (2 matmuls, 2 transposes per tile); "dr" = same but plain DoubleRow with
# a strided 3D weights AP; "fp8chunks" = 4 fp8 chunk matmuls + 4 strided
# fp8 transposes per tile.
R_MODE = "drswi"

_CACHE = {}


def _make_act_root():
    """Build an act-root dir whose act_info.json contains only the
    natural_log_exp_and_others table set (covers Ln/Exp/Relu/Copy/
    Identity) so the ScalarE never switches table sets mid-kernel."""
    import json
    import tempfile

    if os.environ.get("BASS_ACT_ROOT_JSON_PATH"):
        return _CACHE.get("act_root_ours", False)
    try:
        from neuronxcc.driver.Job import Job
        from neuronxcc.driver.jobs.support.FindActInfo import findActInfoFile

        src_json = findActInfoFile(Job.getPackageDir(), "gen3")
        src_dir = os.path.dirname(src_json)
        with open(src_json) as f:
            info = json.load(f)
        keep = [s for s in info.get("act_func_sets", [])
                if s.get("name") == "natural_log_exp_and_others"]
        if not keep:
            return
        info["act_func_sets"] = keep
        tmpdir = tempfile.mkdtemp(prefix="act_root_")
        for fn in os.listdir(src_dir):
            sp = os.path.join(src_dir, fn)
            if os.path.isfile(sp) and fn != os.path.basename(src_json):
                os.symlink(sp, os.path.join(tmpdir, fn))
        dst = os.path.join(tmpdir, "act_info.json")
        with open(dst, "w") as f:
            json.dump(info, f)
        os.environ["BASS_ACT_ROOT_JSON_PATH"] = dst
        _CACHE["act_root_ours"] = True
        return True
    except Exception:
        return False


def _pin_act_tables(enabled):
    """Restrict bass's activation-table choices to the single set our
    trimmed act_info.json exposes, so set id 0 is consistent on both
    sides and the ScalarE never reloads tables mid-kernel."""
    if not enabled:
        return
    import functools
    import concourse.hw_specs as hw_specs
    from concourse import bacc

    if getattr(hw_specs.get_activation_tables, "_pinned", False):
        return
    orig = hw_specs.get_activation_tables

    @functools.cache
    def pinned(module_arch):
        full = orig(module_arch)
        name = "natural_log_exp_and_others"
        return {name: full[name]}

    pinned._pinned = True
    hw_specs.get_activation_tables = pinned
    bacc.get_activation_tables = pinned


def _register_sq_add_reduce():
    """Register a custom DVE op: out = (Src0+Src1)^2, accum_out = sum.
    Computes dist^2 = sum_h (z_h + c_h)^2 in a single Vector pass,
    removing both the PE seed matmul and the ScalarE Square."""
    if "sq_op" in _CACHE:
        return _CACHE["sq_op"]
    try:
        from operator import add as _add

        import concourse.dve_ops as dve_ops
        from concourse.dve_spec import Spec, Src0, Src1, Zero, lower, sq
        from concourse.dve_uop import DveOpSpec

        name = "SQ_ADD_REDUCE_ANT"
        if name in dve_ops._SUB_OPCODE_FOR_NAME:
            op = next(o for o in dve_ops.OPS if o.name == name)
            _CACHE["sq_op"] = op
            return op
        row = max(dve_ops._SUB_OPCODE_FOR_NAME.values()) + 1
        assert row < 0x20, "no free custom-DVE opcode row"

        def _ref(in0, in1, s0, s1, imm2):
            b = (in0.astype(np.float32) + in1.astype(np.float32)) ** 2
            b = b.astype(np.float32)
            return b, b.reshape(b.shape[0], -1).sum(axis=-1, keepdims=True)

        spec = Spec(body=sq(Src0 + Src1), accum=_add, accum_init=Zero,
                    reference=_ref)
        shas = {}
        for ver in ("v3", "v4"):
            try:
                uops = lower(spec, ver=ver)
                shas[ver] = DveOpSpec(name=name, opcode=row, uops=uops,
                                      rd1_en=True).sha(ver)
            except Exception:
                pass
        assert shas, "sq_add_reduce failed to lower"
        op = dve_ops.DveOp(name, spec, subdim=False, uops_sha=shas)
        dve_ops.OPS.append(op)
        dve_ops._SUB_OPCODE_FOR_NAME[name] = row
        dve_ops.CUSTOM_DVE_SPECS[name] = spec
        _CACHE["sq_op"] = op
        return op
    except Exception:
        _CACHE["sq_op"] = None
        return None


def _build_program():
    import concourse.tile as tile
    import concourse.mybir as mybir
    from concourse import bacc
    from contextlib import ExitStack

    f32 = mybir.dt.float32
    fp8 = mybir.dt.float8e4
    u16 = mybir.dt.uint16
    bf16 = mybir.dt.bfloat16
    AF = mybir.ActivationFunctionType
    ALU = mybir.AluOpType
    DR = mybir.MatmulPerfMode.DoubleRow
    DRSWI = mybir.MatmulPerfMode.DoubleRowSwInterleave

    _pin_act_tables(_make_act_root())
    sq_op = _register_sq_add_reduce() if USE_CUSTOM_SQ else None
    assert USE_CUSTOM_SQ or R_MODE == "fp8chunks", \
        "DoubleRow R modes fold +c into the DVE reduce; no seed path"

    nc = bacc.Bacc("TRN2", target_bir_lowering=False, debug=False,
                   num_devices=NCORES)

    # ---- DRAM I/O (per-core shards; all weight transforms host-side) ----
    defect = nc.dram_tensor("defect_embeddings", [BLOC * N, H], f32,
                            kind="ExternalInput").ap()
    r_rows_d = nc.dram_tensor("R_rows", [P, HC * H], fp8,
                              kind="ExternalInput").ap()
    r_pairs_d = nc.dram_tensor("R_pairs", [P, 3 * H], fp8,
                               kind="ExternalInput").ap()
    c_bcast_d = nc.dram_tensor("c_bcast", [P, BLOC * H], fp8,
                               kind="ExternalInput").ap()
    c_rows_d = nc.dram_tensor("c_rows", [1, BLOC * H], fp8,
                              kind="ExternalInput").ap()
    ident8_d = nc.dram_tensor("ident8", [P, P], fp8, kind="ExternalInput").ap()
    perm16_d = nc.dram_tensor("perm16", [P, P], bf16, kind="ExternalInput").ap()
    w1t_d = nc.dram_tensor("W1T", [P, 2 * H], f32, kind="ExternalInput").ap()
    w2t_d = nc.dram_tensor("W2T", [P, OUT], f32, kind="ExternalInput").ap()
    b1c_d = nc.dram_tensor("b1_col", [P, 1], f32, kind="ExternalInput").ap()
    b2r_d = nc.dram_tensor("b2_row", [1, OUT], f32, kind="ExternalInput").ap()
    globt_d = nc.dram_tensor("globT", [P, BLOC * HC], f32,
                             kind="ExternalInput").ap()
    out_d = nc.dram_tensor("out", [1, BLOC * OUT], f32,
                           kind="ExternalOutput").ap()

    cast_eng = {"gpsimd": nc.gpsimd, "vector": nc.vector,
                "scalar": nc.scalar}[CAST_ENGINE]

    with tile.TileContext(nc, num_cores=NCORES) as tc, ExitStack() as ctx:
        consts = ctx.enter_context(tc.tile_pool(name="consts", bufs=1))
        dstream = ctx.enter_context(tc.tile_pool(name="dstream", bufs=6))
        dbatch = ctx.enter_context(tc.tile_pool(name="dbatch", bufs=2))
        dtp = ctx.enter_context(tc.tile_pool(name="dtp", bufs=6))
        sqscr = ctx.enter_context(tc.tile_pool(name="sqscr", bufs=3))
        bstat = ctx.enter_context(tc.tile_pool(name="bstat", bufs=2))
        ps_tp = ctx.enter_context(tc.tile_pool(name="ps_tp", bufs=2, space="PSUM"))
        ps_dist = ctx.enter_context(tc.tile_pool(name="ps_dist", bufs=5, space="PSUM"))
        ps_small = ctx.enter_context(tc.tile_pool(name="ps_small", bufs=1, space="PSUM"))

        # Prefetch the first defect group ahead of the constant loads so
        # the pipeline starts as early as possible.
        if CAST_DMA:
            d_f8_0 = dbatch.tile([P, T * H], fp8, tag="d_f8")
            # per-tile granularity so the first transpose can start as
            # soon as the first 128 defects land
            for ti in range(MB // H):
                nc.gpsimd.dma_start(
                    d_f8_0[:, ti * H:(ti + 1) * H],
                    defect[ti * P:(ti + 1) * P, :])
        else:
            dmb0 = dstream.tile([P, MB], f32, tag="dmb")
            nc.sync.dma_start(
                dmb0[:],
                defect[0:(MB // H) * P, :].rearrange("(a p) h -> p a h", p=P))

        # ---------------- constants ----------------
        ones_f32 = consts.tile([P, P], f32)
        nc.vector.memset(ones_f32[:], 1.0)
        ident_f32 = consts.tile([P, P], f32)
        nc.gpsimd.affine_select(ident_f32[:], ones_f32[:], pattern=[[-1, P]],
                                compare_op=ALU.is_equal, fill=0.0, base=0,
                                channel_multiplier=1)
        ones_f8 = consts.tile([P, 1], fp8)
        nc.vector.memset(ones_f8[:], 1.0)
        ones_row8 = consts.tile([1, P], fp8)
        nc.vector.memset(ones_row8[:], 1.0)
        neg_shift_col = consts.tile([P, 1], f32)
        nc.vector.memset(neg_shift_col[:], -SHIFT)

        # critical-path consts first: the first transposes need perm16 and
        # the first R matmuls need r_pairs.
        if R_MODE == "fp8chunks":
            ident8 = consts.tile([P, P], fp8)
            nc.sync.dma_start(ident8[:], ident8_d[:])
            r_sb = consts.tile([P, HC * H], fp8)
            nc.sync.dma_start(r_sb[:], r_rows_d[:])
        else:
            perm16 = consts.tile([P, P], bf16)
            nc.sync.dma_start(perm16[:], perm16_d[:])
            r_pairs = consts.tile([P, 3 * H], fp8)
            nc.sync.dma_start(r_pairs[:], r_pairs_d[:])
        c_bcast = consts.tile([P, BLOC * H], fp8)
        nc.sync.dma_start(c_bcast[:], c_bcast_d[:])
        c_rows = consts.tile([1, BLOC * H], fp8)
        nc.sync.dma_start(c_rows[:], c_rows_d[:])
        w1t = consts.tile([P, 2 * H], f32)
        nc.sync.dma_start(w1t[:], w1t_d[:])
        w2t = consts.tile([P, OUT], f32)
        nc.sync.dma_start(w2t[:], w2t_d[:])
        b1_col = consts.tile([P, 1], f32)
        nc.sync.dma_start(b1_col[:], b1c_d[:])
        b2_row = consts.tile([1, OUT], f32)
        nc.sync.dma_start(b2_row[:], b2r_d[:])
        globT = consts.tile([P, BLOC * HC], f32)
        nc.sync.dma_start(globT[:], globt_d[:])

        result_sb = consts.tile([1, BLOC * OUT], f32)

        pm = DRSWI if R_MODE == "drswi" else DR
        pending = None

        def emit_r_sq(dT_sw, rb, rt, rsq_cols):
            # z = R d via two K=256 DoubleRow matmuls (chunk of h 256..511
            # covers all 512 z cols; h 0..255 covers the first 256), then
            # dist2 = sum_h (z + c~)^2 in one fused DVE op.
            if R_MODE == "drswi":
                w_hi = dT_sw[:, 2 * P:4 * P]
                w_lo = dT_sw[:, 0:2 * P]
            else:
                w_hi = dT_sw[:, 2 * P:4 * P].rearrange("p (m i) -> p i m", i=2)
                w_lo = dT_sw[:, 0:2 * P].rearrange("p (m i) -> p i m", i=2)
            dist_ps = ps_dist.tile([P, H], f32, tag="dist_ps")
            nc.tensor.matmul(
                dist_ps[:, :], w_hi,
                r_pairs[:, 0:2 * H].rearrange("p (i w) -> p i w", i=2),
                start=True, stop=False, perf_mode=pm)
            nc.tensor.matmul(
                dist_ps[:, :2 * P], w_lo,
                r_pairs[:, 2 * H:3 * H].rearrange("p (i w) -> p i w", i=2),
                start=False, stop=True, perf_mode=pm)
            scr = sqscr.tile([P, H], fp8, tag="scr")
            nc.vector._custom_dve(
                sq_op, out=scr[:], in0=dist_ps[:],
                in1=c_bcast[:, rb * H:(rb + 1) * H],
                accum_out=rsq_cols[:, rt:rt + 1])

        # ---------------- pipelined main loop ----------------
        batch_tiles = {}

        def get_batch(b):
            if b not in batch_tiles:
                if CAST_DMA and b == 0:
                    d_f8 = d_f8_0
                else:
                    d_f8 = dbatch.tile([P, T * H], fp8, tag="d_f8")
                sq_cols = bstat.tile([P, T], f32, tag="sq_cols")
                batch_tiles[b] = (d_f8, sq_cols)
            return batch_tiles[b]

        def emit_front(b, g):
            # DMA + pair transposes + weights copy for one group; the R
            # matmuls + distance reduce trail one tile behind so the PE
            # never stalls on the PSUM->SBUF weights copy.
            nonlocal pending
            d_f8, sq_cols = get_batch(b)
            if not (CAST_DMA and b == 0 and g == 0):
                nc.gpsimd.dma_start(
                    d_f8[:, g * MB:(g + 1) * MB],
                    defect[b * N + g * (MB // H) * P:
                           b * N + (g + 1) * (MB // H) * P, :]
                    .rearrange("(a p) h -> p a h", p=P))
            for ti in range(MB // H):
                t = g * (MB // H) + ti
                d_u16 = d_f8[:, t * H:(t + 1) * H].bitcast(bf16)
                tp16 = ps_tp.tile([P, 2 * P], bf16, tag="tp_ps")
                for c in range(2):
                    nc.tensor.transpose(
                        tp16[:, c * P:(c + 1) * P],
                        d_u16[:, c * P:(c + 1) * P],
                        perm16[:])
                dT_sw = dtp.tile([P, H], fp8, tag="dT")
                if DT_COPY_ENGINE == "scalar":
                    nc.scalar.copy(dT_sw.bitcast(bf16), tp16[:])
                else:
                    nc.vector.tensor_copy(dT_sw.bitcast(bf16), tp16[:])
                if pending is not None:
                    emit_r_sq(*pending)
                pending = (dT_sw, b, t, sq_cols)

        def emit_tail(b, d_f8, sq_cols):
            # ---- softmax stats (constant shift, no cross-tile max) ----
            tln = bstat.tile([P, T], f32, tag="tln")
            nc.scalar.activation(tln[:], sq_cols[:], AF.Ln)
            dist_sb = bstat.tile([P, T], f32, tag="dist_sb")
            nc.scalar.activation(dist_sb[:], tln[:], AF.Exp, scale=0.5)
            e_f32 = bstat.tile([P, T], f32, tag="e_f32")
            nc.scalar.activation(e_f32[:], dist_sb[:], AF.Exp,
                                 bias=neg_shift_col[:])
            # cast e -> fp8 in pair-friendly layout: col j*16+k holds tile
            # t = 2k+j, so a DoubleRow pair k is cols {k, 16+k} (16B stride)
            e_f8 = bstat.tile([P, T], fp8, tag="e_f8")
            if CTX_DOUBLE_ROW:
                nc.scalar.copy(
                    e_f8.rearrange("p (j k) -> p j k", j=2),
                    e_f32.rearrange("p (k j) -> p j k", j=2))
            else:
                nc.scalar.copy(e_f8[:], e_f32[:])

            # S = sum(e): cross-partition sum via a 1-column ones matmul
            s_ps = ps_small.tile([1, T], f32, tag="sm_ps")
            nc.tensor.matmul(s_ps[:, :], ones_f8[:, :1], e_f8[:, :],
                             start=True, stop=True)
            s_sc = bstat.tile([1, 1], f32, tag="s_sc")
            nc.vector.reduce_sum(s_sc[:], s_ps[:], axis=mybir.AxisListType.X)
            recip_s = bstat.tile([1, 1], f32, tag="recip_s")
            nc.vector.reciprocal(recip_s[:], s_sc[:])

            # ---- context = (sum_n e_n d_n) / S ----
            ctx_ps = ps_small.tile([1, H], f32, tag="sm_ps")
            if CTX_DOUBLE_ROW:
                e_pairs = e_f8.rearrange("p (j k) -> p j k", j=2)
                d_pairs = d_f8.rearrange("p (k h) -> p k h", k=T)
                for k in range(T // 2):
                    nc.tensor.matmul(
                        ctx_ps[:, :],
                        e_pairs[:, :, k:k + 1],
                        d_pairs[:, 2 * k:2 * k + 2, :],
                        start=(k == 0), stop=(k == T // 2 - 1),
                        perf_mode=DR)
            else:
                for t in range(T):
                    nc.tensor.matmul(ctx_ps[:, :], e_f8[:, t:t + 1],
                                     d_f8[:, t * H:(t + 1) * H],
                                     start=(t == 0), stop=(t == T - 1))
            context_sb = bstat.tile([1, H], f32, tag="context_sb")
            nc.scalar.activation(context_sb[:], ctx_ps[:], AF.Copy,
                                 scale=recip_s[:1, :1])

            # ---- MLP ----
            tp = ps_small.tile([P, HC], f32, tag="sm_ps")
            for fc in range(HC):
                nc.tensor.transpose(tp[:, fc:fc + 1],
                                    context_sb[:, fc * P:(fc + 1) * P],
                                    ident_f32[:1, :1])
            combT = bstat.tile([P, HC], f32, tag="combT")
            nc.vector.tensor_copy(combT[:], tp[:])

            h1_ps = ps_small.tile([P, 1], f32, tag="sm_ps")
            for fc in range(2 * H // P):
                rhs = (combT[:, fc:fc + 1] if fc < HC
                       else globT[:, b * HC + fc - HC: b * HC + fc - HC + 1])
                nc.tensor.matmul(h1_ps[:, :], w1t[:, fc * P:(fc + 1) * P],
                                 rhs, start=(fc == 0),
                                 stop=(fc == 2 * H // P - 1))
            h1_sb = bstat.tile([P, 1], f32, tag="h1_sb")
            nc.scalar.activation(h1_sb[:], h1_ps[:], AF.Relu, bias=b1_col[:])

            o_ps = ps_small.tile([1, OUT], f32, tag="sm_ps")
            nc.tensor.matmul(o_ps[:, :], h1_sb[:, :], w2t[:, :],
                             start=True, stop=True)
            nc.vector.tensor_add(result_sb[:, b * OUT:(b + 1) * OUT],
                                 o_ps[:], b2_row[:])

        # The next batch's first group is emitted BEFORE this batch's
        # softmax tail, so the PE stream has transpose/R work to chew on
        # while the ScalarE softmax chain runs.
        for g in range(G):
            emit_front(0, g)
        for b in range(BLOC):
            if b + 1 < BLOC:
                emit_front(b + 1, 0)
            elif pending is not None:
                emit_r_sq(*pending)
                pending = None
            d_f8, sq_cols = get_batch(b)
            emit_tail(b, d_f8, sq_cols)
            if b + 1 < BLOC:
                for g in range(1, G):
                    emit_front(b + 1, g)

        nc.sync.dma_start(out_d[:], result_sb[:])

    nc.compile()
    return nc


def _get_program():
    if "nc" not in _CACHE:
        _CACHE["nc"] = _build_program()
    return _CACHE["nc"]


def _host_prep(inputs):
    """Fold every weight-only transform on the host (fp64 for stability)."""
    f32 = np.float32
    wa = np.asarray(inputs["Wa_w"], dtype=np.float64)        # [H, H] (o, h)
    wab = np.asarray(inputs["Wa_b"], dtype=np.float64).reshape(H)
    ua = np.asarray(inputs["Ua_w"], dtype=np.float64)
    uab = np.asarray(inputs["Ua_b"], dtype=np.float64).reshape(H)
    nrm = np.asarray(inputs["normal_embedding"], dtype=np.float64).reshape(B, H)
    gf = np.asarray(inputs["global_features"], dtype=np.float64)  # [B, H]
    w1 = np.asarray(inputs["W1"], dtype=np.float64)          # [MID, 2H]
    b1 = np.asarray(inputs["b1"], dtype=np.float64).reshape(MID)
    w2 = np.asarray(inputs["W2"], dtype=np.float64)          # [OUT, MID]
    b2 = np.asarray(inputs["b2"], dtype=np.float64).reshape(OUT)

    # QR: Wa = Q R  =>  ||Wa d + c|| = ||R d + Q^T c||, R upper-triangular.
    Q, R = np.linalg.qr(wa)
    # pack rhs rows: r_rows[p, j*H + i] = R^T[j*128+p, i] = R[i, j*128+p]
    r_rows = np.zeros((P, HC * H), dtype=np.float64)
    RT = R.T
    for j in range(HC):
        w = (j + 1) * P
        r_rows[:, j * H: j * H + w] = RT[j * P:(j + 1) * P, :w]

    # DoubleRow pair layout: partition p, pair element i carries h = 2p+i
    # (+256 for the hi chunk).  r_pairs[p, i*512+col] = R[col, 256+2p+i];
    # r_pairs[p, 1024+i*256+col] = R[col, 2p+i].
    r_pairs = np.zeros((P, 3 * H), dtype=np.float64)
    hp_idx = 2 * np.arange(P)
    for i in range(2):
        r_pairs[:, i * H:(i + 1) * H] = RT[256 + hp_idx + i, :]
        r_pairs[:, 2 * H + i * 2 * P:2 * H + (i + 1) * 2 * P] = \
            RT[hp_idx + i, :2 * P]

    ua_all = nrm @ ua.T + uab                     # [B, H]
    c_all = wab[None, :] - ua_all                 # [B, H]
    ct_all = c_all @ Q                            # [B, H]  (= (Q^T c)^T)

    w1t = np.zeros((P, 2 * H), dtype=np.float64)
    for fc in range(2 * H // P):
        w1t[:, fc * P:(fc + 1) * P] = w1[:, fc * P:(fc + 1) * P].T

    return {
        "r_rows": r_rows,
        "r_pairs": r_pairs,
        "ct_all": ct_all,
        "gf": gf,
        "w1t": w1t.astype(f32),
        "w2t": np.ascontiguousarray(w2.T).astype(f32),
        "b1_col": b1.reshape(P, 1).astype(f32),
        "b2_row": b2.reshape(1, OUT).astype(f32),
    }


def _make_in_maps(inputs):
    import ml_dtypes

    f32 = np.float32
    f8 = getattr(ml_dtypes, "float8_e4m3", ml_dtypes.float8_e4m3fn)
    hp = _host_prep(inputs)
    d = np.ascontiguousarray(inputs["defect_embeddings"], dtype=f32)
    ident8 = np.eye(P, dtype=np.float64).astype(f8)
    perm = np.eye(P) if R_MODE == "dr" else np.eye(P)[::-1]
    perm16 = np.ascontiguousarray(perm).astype(ml_dtypes.bfloat16)

    in_maps = []
    for c in range(NCORES):
        lo = c * BLOC
        globt = np.zeros((P, BLOC * HC), dtype=np.float64)
        for b in range(BLOC):
            for j in range(HC):
                globt[:, b * HC + j] = hp["gf"][lo + b, j * P:(j + 1) * P]
        ct = hp["ct_all"][lo:lo + BLOC].reshape(1, BLOC * H)
        m = {
            "defect_embeddings": np.ascontiguousarray(
                d[lo:lo + BLOC].reshape(BLOC * N, H)),
            "R_rows": hp["r_rows"].astype(f8),
            "R_pairs": hp["r_pairs"].astype(f8),
            "c_bcast": np.ascontiguousarray(
                np.broadcast_to(ct, (P, BLOC * H))).astype(f8),
            "c_rows": np.ascontiguousarray(ct).astype(f8),
            "ident8": ident8,
            "perm16": perm16,
            "W1T": hp["w1t"],
            "W2T": hp["w2t"],
            "b1_col": hp["b1_col"],
            "b2_row": hp["b2_row"],
            "globT": globt.astype(f32),
        }
        in_maps.append(m)
    return in_maps


def _install_ntff_hook_shim():
    """The agent image's antenv package lacks axon_hooks; recreate it so
    run_bass_kernel_spmd(trace=True) can capture NTFF profiles."""
    import sys
    import types

    try:
        from antenv.axon_hooks import get_axon_ntff_profile_hook  # noqa: F401
        return
    except ImportError:
        pass
    import antenv
    from trn_agent_boot import trn_boot

    so_path = "/opt/axon/libaxon_pjrt.so"
    hook = trn_boot._ntff_profile_via_ctypes(so_path)
    if hook is None:
        raise RuntimeError("libaxon_pjrt.so lacks profile symbols")
    mod = types.ModuleType("antenv.axon_hooks")
    state = {"hook": hook}
    mod.set_axon_ntff_profile_hook = lambda h: state.__setitem__("hook", h)
    mod.get_axon_ntff_profile_hook = lambda: state["hook"]
    sys.modules["antenv.axon_hooks"] = mod
    antenv.axon_hooks = mod


def kernel(**inputs) -> np.ndarray:
    from concourse.bass_utils import run_bass_kernel_spmd

    nc = _get_program()
    in_maps = _make_in_maps(inputs)
    trace = bool(int(os.environ.get("KERNEL_TRACE", "0")))
    if trace:
        try:
            _install_ntff_hook_shim()
        except Exception:
            trace = False
    res = run_bass_kernel_spmd(nc, in_maps, core_ids=list(range(NCORES)),
                               trace=trace)
    if res.exec_time_ns is not None:
        print(f"HW exec time: {res.exec_time_ns} ns")
    out = np.concatenate(
        [res.results[c]["out"].reshape(BLOC, OUT) for c in range(NCORES)],
        axis=0)
    return out.astype(np.float32)
